# revision 1
# baseline (speedup 1.0000x reference)
"""Local (sliding-window) causal attention kernel for Trainium2, 8 NeuronCores.

Problem: nn_LocalAttention (B=2, S=2048, D=1024, nh=16, hd=64, window=256,
topk=0).  q = x @ Wq.T ; k,v = reshaped inputs ; scores masked to the strict
causal band  qi-256 <= kj <= qi-1 ; softmax ; out = (P @ v) heads concat @ Wo.T.

Sharding: data-parallel over (B, S): 8 shards of 512 query rows; each core gets
its key/value halo of 768 rows.  No collectives.

Device layout: everything is computed in "transposed" (feature-major) layout so
no on-device transposes are needed:
  - host passes xT [D, 512], kT [D, 768], Wq.T, Wo.T; v stays natural.
  - qT = WqT.T @ xT                        (PE)
  - ST[kj, qi] = kT_h.T @ qT_h             (PE, banded windows only)
  - ST = exp(ST/8) * bandmask              (ACT + DVE; no max needed, scores~N(0,1))
  - attnT_unnorm[hd, qi], den[qi] = [v_h | 1].T @ ST   (PE, ones-column trick,
      misaligned windows accumulate via PSUM has_written semantics)
  - norm: dens broadcast across partitions via tiny K=1 PE outer product, one
      reciprocal_approx on the pair tile, fused into the PSUM->SBUF copy
  - outT = WoT.T @ attnT_norm              (PE) ; host transposes back.

Matmul inputs are bf16 (PE fp32 moving-operand throughput is ~4x lower), all
accumulation in fp32 PSUM; softmax denominators exact in fp32 up to the bf16
rounding of the broadcast.  Set DTYPE="f32" for the full-precision variant.
"""

import os
import numpy as np

DTYPE = os.environ.get("LA_DTYPE", "bf16")

NCORES = 8
B, S, D = 2, 2048, 1024
NH, HD = 16, 64
ROWS = 512            # query rows per core
HALO = 256            # window size
KROWS = ROWS + HALO   # 768 key rows per core
NKJ = KROWS // 128    # 6 key chunks

# qi-window of each kj-chunk cj: all qi chunks that the band of cj touches.
WIN = [(max(0, 128 * (cj - 2)), min(ROWS, 128 * cj + 128)) for cj in range(NKJ)]
WIDTHS = [hi - lo for lo, hi in WIN]
MOFF = np.concatenate([[0], np.cumsum(WIDTHS)]).astype(int)  # mask col offsets
MTOT = int(MOFF[-1])  # 1536

_prog = None  # cached compiled program


def _build_program(reps=1, phases=(1, 2, 3)):
    from contextlib import ExitStack
    import concourse.tile as tile
    from concourse import bacc, mybir

    f32 = mybir.dt.float32
    DT = mybir.dt.bfloat16 if DTYPE == "bf16" else f32
    nc = bacc.Bacc("TRN2", target_bir_lowering=False, debug=False,
                   enable_asserts=False)

    # register an eps const AP (only 0.0/1.0 are pre-registered) for the
    # denominator guard: recip(0) is undefined in reciprocal_approx_fast.
    EPS = 1e-20
    eps_t = nc.alloc_sbuf_tensor("const-eps", [128, 1], f32)
    nc.gpsimd.memset(eps_t.ap(), EPS)
    nc.const_aps.aps[(f32, EPS)] = eps_t.ap()
    nc.all_engine_barrier()

    d_xT = nc.dram_tensor("xT", [D, ROWS], DT, kind="ExternalInput").ap()
    d_kT = nc.dram_tensor("kT", [D, KROWS], DT, kind="ExternalInput").ap()
    d_va = nc.dram_tensor("va", [KROWS, NH * 65], DT, kind="ExternalInput").ap()
    d_wq = nc.dram_tensor("wqT", [D, D], DT, kind="ExternalInput").ap()
    d_wo = nc.dram_tensor("woT", [D, D], DT, kind="ExternalInput").ap()
    d_msk = nc.dram_tensor("msk", [128, MTOT], DT, kind="ExternalInput").ap()
    d_out = nc.dram_tensor("outT", [D, ROWS], f32, kind="ExternalOutput").ap()

    EXP = mybir.ActivationFunctionType.Exp

    with tile.TileContext(nc) as tc, ExitStack() as ctx:
        pers = ctx.enter_context(tc.tile_pool(name="pers", bufs=1))
        ps_mm = ctx.enter_context(tc.tile_pool(name="psmm", bufs=2, space="PSUM"))
        ps_st = ctx.enter_context(tc.tile_pool(name="psst", bufs=3, space="PSUM"))
        ps_av = ctx.enter_context(tc.tile_pool(name="psav", bufs=3, space="PSUM"))
        st_pool = ctx.enter_context(tc.tile_pool(name="stp", bufs=8))
        kt_pool = ctx.enter_context(tc.tile_pool(name="ktp", bufs=3))
        bc_pool = ctx.enter_context(tc.tile_pool(name="bcp", bufs=2))
        ot_pool = ctx.enter_context(tc.tile_pool(name="otp", bufs=2))
        den_pool = ctx.enter_context(tc.tile_pool(name="denp", bufs=4))

        for rep in range(reps):
            # ---- persistent loads (scheduler overlaps these with qproj) ----
            va_t = []
            for cj in range(NKJ):
                t = pers.tile([128, NH * 65], DT, tag=f"va{cj}", name=f"va{cj}")
                nc.sync.dma_start(out=t[:], in_=d_va[128 * cj:128 * cj + 128, :])
                va_t.append(t)
            msk_t = pers.tile([128, MTOT], DT, tag="msk")
            nc.sync.dma_start(out=msk_t[:], in_=d_msk[:, :])
            wo_t = []
            for t2 in range(8):
                t = pers.tile([128, D], DT, tag=f"wo{t2}", name=f"wo{t2}")
                nc.sync.dma_start(out=t[:], in_=d_wo[128 * t2:128 * t2 + 128, :])
                wo_t.append(t)

            ones64 = pers.tile([1, 64], DT, tag="ones64")
            nc.vector.memset(ones64[:], 1.0)
            attnT = [pers.tile([128, ROWS], DT, tag=f"at{p}", name=f"at{p}")
                     for p in range(8)]
            qT_t = []

            # ---- phase 1: q projection (wq/x tiles freed afterwards) ----
            with tc.tile_pool(name="wqx", bufs=1) as wqx:
                wq_t, x_t = [], []
                for k2 in range(8):
                    t = wqx.tile([128, D], DT, tag=f"wq{k2}", name=f"wq{k2}")
                    nc.sync.dma_start(out=t[:], in_=d_wq[128 * k2:128 * k2 + 128, :])
                    wq_t.append(t)
                for k2 in range(8):
                    t = wqx.tile([128, ROWS], DT, tag=f"x{k2}", name=f"x{k2}")
                    nc.sync.dma_start(out=t[:], in_=d_xT[128 * k2:128 * k2 + 128, :])
                    x_t.append(t)
                for m in range(8):
                    q = pers.tile([128, ROWS], DT, tag=f"qT{m}", name=f"qT{m}")
                    if 1 in phases:
                        ps = ps_mm.tile([128, ROWS], f32, tag="mm", name="ps_mm_t")
                        for k2 in range(8):
                            nc.tensor.matmul(ps[:],
                                             wq_t[k2][:, 128 * m:128 * m + 128],
                                             x_t[k2][:], start=(k2 == 0),
                                             stop=(k2 == 7))
                        nc.scalar.copy(out=q[:], in_=ps[:])
                    else:
                        nc.vector.memset(q[:], 0.01)
                    qT_t.append(q)

            # ---- phase 2: attention per head ----
            for p in range((8 if 2 in phases else 0)):           # head pair
                kt = kt_pool.tile([128, KROWS], DT, tag="kt", name="kt_p")
                nc.sync.dma_start(out=kt[:], in_=d_kT[128 * p:128 * p + 128, :])
                av_pair = []
                for sub in range(2):
                    h = 2 * p + sub
                    qt = qT_t[p]
                    b0 = 64 * sub
                    st_tiles = []
                    for cj in range(NKJ):
                        lo, hi = WIN[cj]
                        w = hi - lo
                        sp = ps_st.tile([128, w], f32, tag="stp", name="sp_st")
                        nc.tensor.matmul(
                            sp[:],
                            kt[b0:b0 + 64, 128 * cj:128 * cj + 128],
                            qt[b0:b0 + 64, lo:hi],
                            start=True, stop=True)
                        ss = st_pool.tile([128, w], DT, tag="st", name="ss_st")
                        nc.scalar.activation(ss[:], sp[:], EXP, scale=0.125)
                        nc.vector.tensor_mul(
                            ss[:], ss[:],
                            msk_t[:, int(MOFF[cj]):int(MOFF[cj]) + w])
                        st_tiles.append(ss)
                    av = ps_av.tile([65, ROWS], f32, tag="av", name="av_ps")
                    for cj in range(NKJ):
                        lo, hi = WIN[cj]
                        nc.tensor.matmul(
                            av[:, lo:hi],
                            va_t[cj][:, 65 * h:65 * h + 65],
                            st_tiles[cj][:],
                            start=(cj == 0), stop=(cj == NKJ - 1),
                            skip_group_check=True)
                    # denominator row (+eps so recip(0) is finite)
                    dh = den_pool.tile([1, ROWS], DT, tag="den", name="den_h")
                    nc.scalar.add(dh[:], av[64:65, :], EPS)
                    av_pair.append((av, dh))
                # normalization: broadcast dens across 64 partitions per head
                # via K=1 outer product, then one recip over the pair tile.
                bc_ps = ps_mm.tile([128, ROWS], f32, tag="mm", name="ps_mm_t")
                for sub in range(2):
                    nc.tensor.matmul(bc_ps[64 * sub:64 * sub + 64, :], ones64[:],
                                     av_pair[sub][1][:], start=True, stop=True)
                bc_sb = bc_pool.tile([128, ROWS], f32, tag="bc", name="bc_sb")
                nc.vector.reciprocal_approx_fast(out=bc_sb[:], in_=bc_ps[:])
                for sub in range(2):
                    nc.vector.tensor_mul(
                        attnT[p][64 * sub:64 * sub + 64, :],
                        av_pair[sub][0][0:64, :],
                        bc_sb[64 * sub:64 * sub + 64, :])

            if 2 not in phases:
                for p2x in range(8):
                    nc.vector.memset(attnT[p2x][:], 0.01)
            # ---- phase 3: output projection ----
            for n in range((8 if 3 in phases else 0)):
                ps = ps_mm.tile([128, ROWS], f32, tag="mm", name="ps_mm_t")
                for t2 in range(8):
                    nc.tensor.matmul(ps[:], wo_t[t2][:, 128 * n:128 * n + 128],
                                     attnT[t2][:], start=(t2 == 0), stop=(t2 == 7))
                ot = ot_pool.tile([128, ROWS], f32, tag="ot", name="ot_sb")
                nc.vector.tensor_copy(ot[:], ps[:])
                nc.sync.dma_start(out=d_out[128 * n:128 * n + 128, :], in_=ot[:])

    nc.compile()
    return nc


def _to_dt(a):
    if DTYPE == "bf16":
        import ml_dtypes
        return np.ascontiguousarray(a).astype(ml_dtypes.bfloat16)
    return np.ascontiguousarray(a).astype(np.float32)


def _host_prep(query_seq, keys_seq, values_seq, Wq, Wo):
    """Build the 8 per-core input maps."""
    qT_all = np.ascontiguousarray(query_seq.transpose(0, 2, 1))  # [B, D, S]
    kT_all = np.ascontiguousarray(keys_seq.transpose(0, 2, 1))
    wqT = _to_dt(Wq.T)
    woT = _to_dt(Wo.T)

    def band_mask(first):
        m = np.zeros((128, MTOT), np.float32)
        for cj in range(NKJ):
            lo, hi = WIN[cj]
            kj = 128 * cj + np.arange(128)[:, None]
            qi = np.arange(lo, hi)[None, :]
            valid = (kj >= qi) & (kj <= qi + HALO - 1)
            if first:
                valid &= (kj >= HALO)
            m[:, MOFF[cj]:MOFF[cj + 1]] = valid.astype(np.float32)
        return m

    msk_first = _to_dt(band_mask(True))
    msk_rest = _to_dt(band_mask(False))

    in_maps = []
    for c in range(NCORES):
        b, ch = c // 4, c % 4
        r0 = ch * ROWS
        xT = _to_dt(qT_all[b][:, r0:r0 + ROWS])
        kT = np.zeros((D, KROWS), np.float32)
        va = np.zeros((KROWS, NH * 65), np.float32)
        va[:, 64::65] = 1.0  # ones column per head
        if ch == 0:
            kT[:, HALO:] = kT_all[b][:, 0:ROWS]
            v_halo = values_seq[b, 0:ROWS]
            va[HALO:, :] = np.concatenate(
                [v_halo.reshape(ROWS, NH, HD),
                 np.ones((ROWS, NH, 1), np.float32)], axis=2).reshape(ROWS, -1)
        else:
            kT[:, :] = kT_all[b][:, r0 - HALO:r0 + ROWS]
            v_halo = values_seq[b, r0 - HALO:r0 + ROWS]
            va[:, :] = np.concatenate(
                [v_halo.reshape(KROWS, NH, HD),
                 np.ones((KROWS, NH, 1), np.float32)], axis=2).reshape(KROWS, -1)
        in_maps.append({
            "xT": xT, "kT": _to_dt(kT), "va": _to_dt(va), "wqT": wqT,
            "woT": woT, "msk": msk_first if ch == 0 else msk_rest,
        })
    return in_maps


def _run(inputs, trace=False):
    global _prog
    from concourse.bass_utils import run_bass_kernel_spmd

    query_seq = np.asarray(inputs["query_seq"], np.float32)
    keys_seq = np.asarray(inputs["keys_seq"], np.float32)
    values_seq = np.asarray(inputs["values_seq"], np.float32)
    Wq = np.asarray(inputs["Wq"], np.float32)
    Wo = np.asarray(inputs["Wo"], np.float32)
    assert int(inputs.get("window", HALO)) == HALO
    assert int(inputs.get("topk", 0)) == 0

    if _prog is None:
        _prog = _build_program()

    in_maps = _host_prep(query_seq, keys_seq, values_seq, Wq, Wo)
    res = run_bass_kernel_spmd(_prog, in_maps, list(range(NCORES)), trace=trace)

    out = np.empty((B, S, D), np.float32)
    for c in range(NCORES):
        b, ch = c // 4, c % 4
        r0 = ch * ROWS
        out[b, r0:r0 + ROWS, :] = res.results[c]["outT"].T
    return out, res


def kernel(**inputs):
    out, _ = _run(inputs)
    return out



# revision 7
# speedup vs baseline: 18.2589x; 18.2589x over previous
"""Local (sliding-window) causal attention kernel for Trainium2, 8 NeuronCores.

Problem: nn_LocalAttention (B=2, S=2048, D=1024, nh=16, hd=64, window=256,
topk=0).  q = x @ Wq.T ; k,v = reshaped inputs ; scores masked to the strict
causal band  qi-256 <= kj <= qi-1 ; softmax ; out = (P @ v) heads concat @ Wo.T.

Sharding: data-parallel over (B, S): 8 shards of 512 query rows; each core gets
its key/value halo of 768 rows.  No collectives.

Device layout: everything is computed in "transposed" (feature-major) layout so
no on-device transposes are needed:
  - host passes x8 [D, 512] (fp8), kT [D, 768], 16*Wq.T (fp8), Wo.T; v natural.
  - qT = Wq8T.T @ x8T via fp8 DoubleRow matmuls (K_eff=256/mm)      (PE)
  - ST[kj, qi] = kT_h.T @ qT_h   (PE, banded windows, head pairs run
      concurrently on disjoint 64-row PE tiles)
  - ST = exp(ST/(8*16)) * bandmask   (ACT exp with merged 2-bank chunk-pair
      tiles; mask-mul split between DVE and the otherwise-idle GPSIMD)
  - attnT_unnorm[hd, qi], den[qi] = [v_h | 1].T @ ST   (PE, ones-column trick,
      misaligned windows accumulate via PSUM has_written semantics)
  - norm: dens broadcast across partitions via K=1 PE outer product (the two
      heads of a pair use disjoint 64-col PE tiles), one reciprocal_approx on
      the pair tile, fused into the PSUM->SBUF copy
  - outT = WoT.T @ attnT_norm              (PE) ; host transposes back.

Matmul inputs are bf16 (PE fp32 moving-operand throughput is ~4x lower) except
the q-projection which runs fp8e4 DoubleRow (2 fp8 weights per PE cell); all
accumulation is fp32 PSUM.  The Wq scale *16 keeps fp8 wq out of the subnormal
range; the 1/16 is folded into the exp() scale.

The first sequence row has no valid keys; its denominator comes from a
synthetic va ones-column entry on the zero-padded halo row (attn row = 0).
"""

import os
import numpy as np

NCORES = 8
B, S, D = 2, 2048, 1024
NH, HD = 16, 64
ROWS = 512            # query rows per core
HALO = 256            # window size
KROWS = ROWS + HALO   # 768 key rows per core
NKJ = KROWS // 128    # 6 key chunks

# qi-window of each kj-chunk cj: all qi chunks that the band of cj touches.
WIN = [(max(0, 128 * (cj - 2)), min(ROWS, 128 * cj + 128)) for cj in range(NKJ)]
WIDTHS = [hi - lo for lo, hi in WIN]
# chunk pairs (cj, 5-cj) have equal widths; their score tiles share one
# 2-bank PSUM tensor so exp/mask run as single wide instructions.
PAIRS = [(0, 5), (1, 4), (2, 3)]
PW = [WIDTHS[a] for a, _ in PAIRS]               # 128, 256, 384
PMOFF = np.concatenate([[0], np.cumsum([2 * w for w in PW])]).astype(int)
MTOT = int(PMOFF[-1])  # 1536

P1MODE = os.environ.get("LA_P1", "bf16")   # "bf16" | "fp8dr"
QSCALE = 16.0 if P1MODE == "fp8dr" else 1.0  # folded into exp scale

_prog = None  # cached compiled program


def _build_program(reps=1, phases=(1, 2, 3)):
    from contextlib import ExitStack
    import concourse.tile as tile
    from concourse import bacc, mybir

    f32 = mybir.dt.float32
    fp8 = mybir.dt.float8e4
    DT = mybir.dt.bfloat16
    DR = mybir.MatmulPerfMode.DoubleRow
    nc = bacc.Bacc("TRN2", target_bir_lowering=False, debug=False,
                   enable_asserts=False)

    XDT = fp8 if P1MODE == "fp8dr" else DT
    d_x8 = nc.dram_tensor("x8", [D, ROWS], XDT, kind="ExternalInput").ap()
    d_w8 = nc.dram_tensor("wq8", [D, D], XDT, kind="ExternalInput").ap()
    d_kT = nc.dram_tensor("kT", [D, KROWS], DT, kind="ExternalInput").ap()
    d_va = nc.dram_tensor("va", [KROWS, NH * 65], DT, kind="ExternalInput").ap()
    d_wo = nc.dram_tensor("woT", [D, D], DT, kind="ExternalInput").ap()
    d_msk = nc.dram_tensor("msk", [128, MTOT], DT, kind="ExternalInput").ap()
    d_out = nc.dram_tensor("outT", [D, ROWS], f32, kind="ExternalOutput").ap()

    EXP = mybir.ActivationFunctionType.Exp

    with tile.TileContext(nc) as tc, ExitStack() as ctx:
        pers = ctx.enter_context(tc.tile_pool(name="pers", bufs=1))
        ps_mm = ctx.enter_context(tc.tile_pool(name="psmm", bufs=2, space="PSUM"))
        ps_st = ctx.enter_context(tc.tile_pool(name="psst", bufs=2, space="PSUM"))
        ps_av = ctx.enter_context(tc.tile_pool(name="psav", bufs=2, space="PSUM"))
        st_pool = ctx.enter_context(tc.tile_pool(name="stp", bufs=6))
        kt_pool = ctx.enter_context(tc.tile_pool(name="ktp", bufs=3))
        bc_pool = ctx.enter_context(tc.tile_pool(name="bcp", bufs=2))
        ot_pool = ctx.enter_context(tc.tile_pool(name="otp", bufs=2))
        den_pool = ctx.enter_context(tc.tile_pool(name="denp", bufs=4))

        for rep in range(reps):
            # ---- persistent loads (scheduler overlaps these with qproj) ----
            va_t = []
            for cj in range(NKJ):
                t = pers.tile([128, NH * 65], DT, tag=f"va{cj}", name=f"va{cj}")
                nc.sync.dma_start(out=t[:], in_=d_va[128 * cj:128 * cj + 128, :])
                va_t.append(t)
            msk_t = pers.tile([128, MTOT], DT, tag="msk")
            nc.sync.dma_start(out=msk_t[:], in_=d_msk[:, :])
            wo_t = []
            for t2 in range(8):
                t = pers.tile([128, D], DT, tag=f"wo{t2}", name=f"wo{t2}")
                nc.sync.dma_start(out=t[:], in_=d_wo[128 * t2:128 * t2 + 128, :])
                wo_t.append(t)

            ones64 = pers.tile([1, 64], DT, tag="ones64")
            nc.vector.memset(ones64[:], 1.0)
            attnT = [pers.tile([128, ROWS], DT, tag=f"at{p}", name=f"at{p}")
                     for p in range(8)]
            qT_t = []

            # ---- phase 1: q projection (fp8 DoubleRow or bf16) ----
            with tc.tile_pool(name="wqx", bufs=1) as wqx:
                w8 = wqx.tile([128, 8, D], XDT, tag="w8", name="w8")
                x8 = wqx.tile([128, 8, ROWS], XDT, tag="x8", name="x8")
                for t2 in range(8):
                    nc.sync.dma_start(out=w8[:, t2, :],
                                      in_=d_w8[128 * t2:128 * t2 + 128, :])
                    nc.sync.dma_start(out=x8[:, t2, :],
                                      in_=d_x8[128 * t2:128 * t2 + 128, :])
                for m in range(8):
                    q = pers.tile([128, ROWS], DT, tag=f"qT{m}", name=f"qT{m}")
                    if 1 in phases:
                        ps = ps_mm.tile([128, ROWS], f32, tag="mm", name="ps_mm_t")
                        if P1MODE == "fp8dr":
                            for c in range(4):
                                nc.tensor.matmul(
                                    ps[:],
                                    w8[:, 2 * c:2 * c + 2, 128 * m:128 * m + 128],
                                    x8[:, 2 * c:2 * c + 2, :],
                                    start=(c == 0), stop=(c == 3), perf_mode=DR)
                        else:
                            for c in range(8):
                                nc.tensor.matmul(
                                    ps[:], w8[:, c, 128 * m:128 * m + 128],
                                    x8[:, c, :], start=(c == 0), stop=(c == 7))
                        nc.vector.tensor_copy(q[:], ps[:])
                    else:
                        nc.vector.memset(q[:], 0.01)
                    qT_t.append(q)

            # ---- phase 2: attention per head pair ----
            for p in range((8 if 2 in phases else 0)):
                kt = kt_pool.tile([128, KROWS], DT, tag="kt", name="kt_p")
                nc.sync.dma_start(out=kt[:], in_=d_kT[128 * p:128 * p + 128, :])
                av_pair = []
                for sub in range(2):
                    h = 2 * p + sub
                    qt = qT_t[p]
                    b0 = 64 * sub
                    ss_tiles = []
                    for P, (ca, cb) in enumerate(PAIRS):
                        w = PW[P]
                        sp = ps_st.tile([128, 2, 512], f32, tag="stp",
                                        name="sp_st")
                        for j, cj in enumerate((ca, cb)):
                            lo, hi = WIN[cj]
                            nc.tensor.matmul(
                                sp[:, j, :w],
                                kt[b0:b0 + 64, 128 * cj:128 * cj + 128],
                                qt[b0:b0 + 64, lo:hi],
                                start=True, stop=True)
                        ss = st_pool.tile([128, 2, 512], DT, tag="st",
                                          name="ss_st")
                        nc.scalar.activation(ss[:, :, :w], sp[:, :, :w], EXP,
                                             scale=0.125 / QSCALE)
                        mop = int(PMOFF[P])
                        mask_ap = msk_t[:, mop:mop + 2 * w].rearrange(
                            "p (j w) -> p j w", j=2)
                        eng = nc.gpsimd if P == 2 else nc.vector
                        eng.tensor_mul(ss[:, :, :w], ss[:, :, :w], mask_ap)
                        ss_tiles.append(ss)
                    av = ps_av.tile([65, ROWS], f32, tag="av", name="av_ps")
                    first = True
                    for P, (ca, cb) in enumerate(PAIRS):
                        w = PW[P]
                        for j, cj in enumerate((ca, cb)):
                            lo, hi = WIN[cj]
                            nc.tensor.matmul(
                                av[:, lo:hi],
                                va_t[cj][:, 65 * h:65 * h + 65],
                                ss_tiles[P][:, j, :w],
                                start=first, stop=(P == 2 and j == 1),
                                skip_group_check=True)
                            first = False
                    dh = den_pool.tile([1, ROWS], DT, tag="den", name="den_h")
                    nc.scalar.copy(dh[:], av[64:65, :])
                    av_pair.append((av, dh))
                # normalization: broadcast dens across 64 partitions per head
                # via K=1 outer product, then one recip over the pair tile.
                bc_ps = ps_mm.tile([128, ROWS], f32, tag="mm", name="ps_mm_t")
                for sub in range(2):
                    nc.tensor.matmul(bc_ps[64 * sub:64 * sub + 64, :], ones64[:],
                                     av_pair[sub][1][:], start=True, stop=True)
                bc_sb = bc_pool.tile([128, ROWS], f32, tag="bc", name="bc_sb")
                nc.vector.reciprocal_approx_fast(out=bc_sb[:], in_=bc_ps[:])
                for sub in range(2):
                    nc.vector.tensor_mul(
                        attnT[p][64 * sub:64 * sub + 64, :],
                        av_pair[sub][0][0:64, :],
                        bc_sb[64 * sub:64 * sub + 64, :])

            if 2 not in phases:
                for p2x in range(8):
                    nc.vector.memset(attnT[p2x][:], 0.01)
            # ---- phase 3: output projection ----
            for n in range((8 if 3 in phases else 0)):
                ps = ps_mm.tile([128, ROWS], f32, tag="mm", name="ps_mm_t")
                for t2 in range(8):
                    nc.tensor.matmul(ps[:], wo_t[t2][:, 128 * n:128 * n + 128],
                                     attnT[t2][:], start=(t2 == 0), stop=(t2 == 7))
                ot = ot_pool.tile([128, ROWS], f32, tag="ot", name="ot_sb")
                nc.vector.tensor_copy(ot[:], ps[:])
                nc.sync.dma_start(out=d_out[128 * n:128 * n + 128, :], in_=ot[:])

    nc.compile()
    return nc


def _to_bf(a):
    import ml_dtypes
    return np.ascontiguousarray(a).astype(ml_dtypes.bfloat16)


def _to_f8(a):
    import ml_dtypes
    return np.ascontiguousarray(a).astype(ml_dtypes.float8_e4m3)


def _band_mask():
    """Uniform band mask in chunk-pair layout: valid iff qi <= kj <= qi+255
    (local coords; halo padding is handled by zeroed va rows, not the mask)."""
    m = np.zeros((128, MTOT), np.float32)
    for P, pair in enumerate(PAIRS):
        w = PW[P]
        for j, cj in enumerate(pair):
            lo, hi = WIN[cj]
            kj = 128 * cj + np.arange(128)[:, None]
            qi = np.arange(lo, hi)[None, :]
            valid = (kj >= qi) & (kj <= qi + HALO - 1)
            c0 = int(PMOFF[P]) + j * w
            m[:, c0:c0 + w] = valid.astype(np.float32)
    return m


def _host_prep(query_seq, keys_seq, values_seq, Wq, Wo):
    """Build the 8 per-core input maps."""
    qT_all = np.ascontiguousarray(query_seq.transpose(0, 2, 1))  # [B, D, S]
    kT_all = np.ascontiguousarray(keys_seq.transpose(0, 2, 1))
    _to_x = _to_f8 if P1MODE == "fp8dr" else _to_bf
    wq8 = _to_x(Wq.T * QSCALE)
    woT = _to_bf(Wo.T)
    msk = _to_bf(_band_mask())

    in_maps = []
    for c in range(NCORES):
        b, ch = c // 4, c % 4
        r0 = ch * ROWS
        x8 = _to_x(qT_all[b][:, r0:r0 + ROWS])
        kT = np.zeros((D, KROWS), np.float32)
        va = np.zeros((KROWS, NH * 65), np.float32)
        va[:, 64::65] = 1.0  # ones column per head
        if ch == 0:
            kT[:, HALO:] = kT_all[b][:, 0:ROWS]
            v_halo = values_seq[b, 0:ROWS]
            va[1:HALO, 64::65] = 0.0  # halo padding: no den contribution
            # row 0 keeps ones=1: den guard for the empty qi=0 window
            va[HALO:, :] = np.concatenate(
                [v_halo.reshape(ROWS, NH, HD),
                 np.ones((ROWS, NH, 1), np.float32)], axis=2).reshape(ROWS, -1)
        else:
            kT[:, :] = kT_all[b][:, r0 - HALO:r0 + ROWS]
            v_halo = values_seq[b, r0 - HALO:r0 + ROWS]
            va[:, :] = np.concatenate(
                [v_halo.reshape(KROWS, NH, HD),
                 np.ones((KROWS, NH, 1), np.float32)], axis=2).reshape(KROWS, -1)
        in_maps.append({
            "x8": x8, "kT": _to_bf(kT), "va": _to_bf(va), "wq8": wq8,
            "woT": woT, "msk": msk,
        })
    return in_maps


def _run(inputs, trace=False):
    global _prog
    from concourse.bass_utils import run_bass_kernel_spmd

    query_seq = np.asarray(inputs["query_seq"], np.float32)
    keys_seq = np.asarray(inputs["keys_seq"], np.float32)
    values_seq = np.asarray(inputs["values_seq"], np.float32)
    Wq = np.asarray(inputs["Wq"], np.float32)
    Wo = np.asarray(inputs["Wo"], np.float32)
    assert int(inputs.get("window", HALO)) == HALO
    assert int(inputs.get("topk", 0)) == 0

    if _prog is None:
        _prog = _build_program()

    in_maps = _host_prep(query_seq, keys_seq, values_seq, Wq, Wo)
    res = run_bass_kernel_spmd(_prog, in_maps, list(range(NCORES)), trace=trace)

    out = np.empty((B, S, D), np.float32)
    for c in range(NCORES):
        b, ch = c // 4, c % 4
        r0 = ch * ROWS
        out[b, r0:r0 + ROWS, :] = res.results[c]["outT"].T
    return out, res


def kernel(**inputs):
    out, _ = _run(inputs)
    return out
